# revision 7
# baseline (speedup 1.0000x reference)
"""Trainium2 Bass kernel for nn_BRepFaceEncoder (gnn_message_passing).

Sharding: the 60000 faces are split contiguously across 8 NeuronCores. Each
core back-chains the halo it needs (faces -> loops -> edges -> vertices) and
runs the whole pipeline locally - no collectives.

Math identities used:
  segment_max_d(x_dst[d] - x_src[s]) == x_dst[d] - segment_min_s(x_src[s])
  min(leaky(z)) == leaky(min(z))   (monotone; exact - conv1 only)
  concat([x, x - m]) @ Wc == x @ (A + B) + m @ (-B)   (A=Wc[:H], B=Wc[H:])

All compute in bf16 (PE matmul 1 cyc/row vs 4 for fp32; DVE 2x on 16-bit),
fp32 PSUM accumulation. conv1 needs no gather: raw vertex positions are
host-staged into per-round slot order and min-accumulated in pre-activation
space. conv2/3 gather previous-layer rows with gpsimd dma_gather ops
(HW-verified exact; multi-index indirect_dma_start is NOT - the firmware
reads only offset[p,0] and strides rows contiguously). dma_gather indices
are int16, so conv1's output table is split into <=32766-row "classes":
conv2's blocks are greedily partitioned into contiguous ranges whose unique
sources fit a class; edges used by several classes are duplicated into each
(few % extra conv1 compute). Each class is its own DRAM tensor, which also
gives the tile scheduler class-granular write->gather dependencies.

Destinations are degree-sorted into 128-row blocks bucketed by round count R.
Leaky runs as a single native Lrelu op on the Act engine (alpha=0.01,
HW-verified); conv1's min accumulates directly in transposed space via
wv-chunk-as-lhsT matmuls (first two rounds min directly off PSUM, no Act
copy), so no PE transposes or bridge copies are needed there.
"""

import sys
from contextlib import ExitStack

import numpy as np
import ml_dtypes

if "/opt/trn_rl_repo" not in sys.path:
    sys.path.insert(0, "/opt/trn_rl_repo")

import concourse.bass as bass            # noqa: E402
import concourse.tile as tile            # noqa: E402
from concourse import bacc, mybir, library_config  # noqa: E402
from concourse.bass_utils import run_bass_kernel_spmd  # noqa: E402
from concourse.masks import make_identity              # noqa: E402

f32 = mybir.dt.float32
bf16 = mybir.dt.bfloat16
i16 = mybir.dt.int16
ALU = mybir.AluOpType
NPBF = ml_dtypes.bfloat16

H = 256
C = 8
BIG = np.float32(512.0)
BUCKETS = (1, 2, 3, 4, 5, 6, 7, 8, 9, 10, 12, 14, 16, 20, 24, 32)
GMAP = {1: 4, 2: 4, 3: 4, 4: 4, 5: 2, 6: 2, 7: 2, 8: 2, 9: 1, 10: 1,
        12: 1, 14: 1, 16: 1, 20: 1, 24: 1, 32: 1}
CLASS_CAP = 30500          # max unique sources per e1 class (idx fits int16)
GCOLS = 16                 # target grid columns per dma_gather op


# ==========================================================================
# Host-side schedule construction
# ==========================================================================

def _build_conv_schedule(dloc, sloc, n_dst):
    """Blocks of 128 degree-sorted dsts, bucketed by round count R."""
    counts = np.bincount(dloc, minlength=n_dst)
    order_p = np.argsort(dloc, kind="stable")
    srcs_sorted = sloc[order_p]
    starts = np.zeros(n_dst + 1, dtype=np.int64)
    np.cumsum(counts, out=starts[1:])

    perm = np.argsort(-counts, kind="stable")
    n_blk = (n_dst + 127) // 128
    pad = n_blk * 128 - n_dst
    perm_padded = np.concatenate([perm, np.full(pad, perm[-1] if n_dst else 0,
                                                dtype=perm.dtype)])
    deg_padded = counts[perm_padded]
    deg_padded[n_dst:] = 0

    bucket_blocks = {}
    for b in range(n_blk):
        dsts = perm_padded[b * 128:(b + 1) * 128]
        degs = deg_padded[b * 128:(b + 1) * 128]
        mx = int(degs[0])
        R = next(r for r in BUCKETS if r >= max(mx, 1))
        slots = np.full((128, R), -1, dtype=np.int64)
        base = starts[dsts]
        for r in range(R):
            have = degs > r
            if have.any():
                slots[have, r] = srcs_sorted[base[have] + r]
        d, s = bucket_blocks.setdefault(R, ([], []))
        d.append(dsts)
        s.append(slots)
    return {R: (np.stack(d), np.stack(s)) for R, (d, s) in bucket_blocks.items()}


def _split_classes(sch, cap):
    """Greedy partition of blocks (group granularity, layout order) into
    classes whose unique source sets stay <= cap. Returns a list of
    per-class schedules ({R: (d, s)}) and per-class sorted unique sources."""
    classes = []
    cur = {}
    cur_set = set()

    def flush():
        nonlocal cur, cur_set
        if cur:
            sch_c = {R: (np.stack(d), np.stack(s)) for R, (d, s) in cur.items()}
            srcs = np.array(sorted(cur_set), dtype=np.int64)
            classes.append((sch_c, srcs))
        cur, cur_set = {}, set()

    for R in BUCKETS:
        if R not in sch:
            continue
        d_all, s_all = sch[R]
        g = GMAP[R]
        nb = d_all.shape[0]
        for g0 in range(0, nb, g):
            blk_d = d_all[g0:g0 + g]
            blk_s = s_all[g0:g0 + g]
            srcs = set(blk_s[blk_s >= 0].tolist())
            if cur_set and len(cur_set | srcs) > cap:
                flush()
            dd, ss = cur.setdefault(R, ([], []))
            for b in range(blk_d.shape[0]):
                dd.append(blk_d[b])
                ss.append(blk_s[b])
            cur_set |= srcs
    flush()
    return classes


def _host_prep(inputs):
    e2v = np.asarray(inputs["edge_to_vertex"])
    l2e = np.asarray(inputs["loop_to_edge"])
    f2l = np.asarray(inputs["face_to_loop"])

    NV = inputs["vertex_positions"].shape[0]
    NE = inputs["edge_curves"].shape[0]
    NL = inputs["loop_types"].shape[0]
    NF = inputs["face_surfaces"].shape[0]

    pos = np.asarray(inputs["vertex_positions"], np.float32)
    raw_feats = [
        np.concatenate([np.asarray(inputs["edge_curves"], np.float32),
                        np.asarray(inputs["edge_curve_parameters"], np.float32),
                        np.asarray(inputs["edge_curve_flipped"], np.float32)[:, None]], axis=1),
        np.asarray(inputs["loop_types"], np.float32),
        np.concatenate([np.asarray(inputs["face_surfaces"], np.float32),
                        np.asarray(inputs["face_surface_parameters"], np.float32),
                        np.asarray(inputs["face_surface_flipped"], np.float32)[:, None]], axis=1),
    ]

    cores = []
    for i in range(C):
        lo, hi = i * NF // C, (i + 1) * NF // C
        mask = np.zeros(NF, bool); mask[lo:hi] = True
        m3 = mask[f2l[0]]
        d3, s3 = f2l[0][m3] - lo, f2l[1][m3]
        loops_i = np.unique(s3)
        mask = np.zeros(NL, bool); mask[loops_i] = True
        m2 = mask[l2e[0]]
        d2, s2 = l2e[0][m2], l2e[1][m2]
        edges_i = np.unique(s2)
        mask = np.zeros(NE, bool); mask[edges_i] = True
        m1 = mask[e2v[0]]
        d1, s1 = e2v[0][m1], e2v[1][m1]
        verts_i = np.unique(s1)

        d1l = np.searchsorted(edges_i, d1)
        s1l = np.searchsorted(verts_i, s1)
        d2l = np.searchsorted(loops_i, d2)
        s2l = np.searchsorted(edges_i, s2)
        s3l = np.searchsorted(loops_i, s3)

        sch2_full = _build_conv_schedule(d2l, s2l, len(loops_i))
        sch3 = _build_conv_schedule(d3, s3l, hi - lo)

        # class split of conv2 blocks by unique edge sources
        cls2 = _split_classes(sch2_full, CLASS_CAP)

        # conv1 sub-schedule per class (over that class's edge copies)
        sch1_c, edges_c = [], []
        for _, srcs in cls2:
            mark = np.zeros(len(edges_i), bool)
            mark[srcs] = True
            selp = mark[d1l]
            d1c = np.searchsorted(srcs, d1l[selp])
            sch1_c.append(_build_conv_schedule(d1c, s1l[selp], len(srcs)))
            edges_c.append(srcs)

        cores.append(dict(lo=lo, hi=hi, loops=loops_i, edges=edges_i,
                          verts=verts_i, sch1_c=sch1_c, edges_c=edges_c,
                          cls2=[s for s, _ in cls2], sch3=sch3))

    NSEG1 = max(len(c["sch1_c"]) for c in cores)
    NVp = ((max(len(c["verts"]) for c in cores) + 511) // 512) * 512

    # global padded block counts per (k, class, R); k=0 classes = NSEG1
    def sub_schedules(c, k):
        if k == 0:
            return c["sch1_c"] + [{}] * (NSEG1 - len(c["sch1_c"]))
        if k == 1:
            return c["cls2"] + [{}] * (NSEG1 - len(c["cls2"]))
        return [c["sch3"]]

    nseg = [NSEG1, NSEG1, 1]
    bucket_counts = [{}, {}, {}]
    for k in range(3):
        for c in cores:
            for ci, sch in enumerate(sub_schedules(c, k)):
                for R, (d, s) in sch.items():
                    g = GMAP[R]
                    n = -(-d.shape[0] // g) * g
                    key = (ci, R)
                    bucket_counts[k][key] = max(bucket_counts[k].get(key, 0), n)
    nblk = [sum(bucket_counts[k].values()) for k in range(3)]

    # e1 class tensor sizes (rows, +1 dummy each); l2 single table
    e1_rows = []
    for ci in range(NSEG1):
        n = sum(nb for (cc, R), nb in bucket_counts[0].items() if cc == ci) * 128
        assert n + 1 <= 32767, f"e1 class {ci} too big: {n}"
        e1_rows.append(n)
    l2_rows = nblk[1] * 128
    assert l2_rows + 1 <= 32767, f"l2 too big: {l2_rows}"

    meta = dict(NVp=NVp, bucket_counts=bucket_counts, nblk=nblk, nseg=nseg,
                e1_rows=e1_rows, l2_rows=l2_rows, F=[16, 11, 18])

    # layout bases: global column base per (k, ci, R); local row base for e1
    col_bases = [{}, {}, {}]     # rawT column base (in blocks)
    row_bases = [{}, {}, {}]     # output row base (local to class tensor for k=0)
    for k in range(3):
        cb = 0
        rb_cls = {}
        for ci in range(nseg[k]):
            for R in BUCKETS:
                nb = bucket_counts[k].get((ci, R), 0)
                if nb == 0:
                    continue
                col_bases[k][(ci, R)] = cb
                if k == 0:
                    row_bases[k][(ci, R)] = rb_cls.get(ci, 0)
                    rb_cls[ci] = rb_cls.get(ci, 0) + nb * 128
                else:
                    row_bases[k][(ci, R)] = cb * 128
                cb += nb
    meta["col_bases"] = col_bases
    meta["row_bases"] = row_bases

    # grid-column bases per (k, ci, R) for gather index arrays (k=1,2),
    # local to the (k, ci) index tensor
    gcol_bases = [None, {}, {}]
    gcols_tot = [None, {}, {}]
    for k in (1, 2):
        for ci in range(nseg[k]):
            gc = 0
            for R in BUCKETS:
                nb = bucket_counts[k].get((ci, R), 0)
                if nb == 0:
                    continue
                gcol_bases[k][(ci, R)] = gc
                gc += nb * R
            gcols_tot[k][ci] = gc
    meta["gcol_bases"] = gcol_bases
    meta["gcols_tot"] = gcols_tot

    def wrap16(flat):
        """[n*128] int array -> dma_gather wrapped [128, n*8] int16 layout."""
        n = flat.shape[0] // 16
        w = np.empty((128, n), np.int16)
        blk = flat.reshape(n, 16).T.astype(np.int16)   # [16, n]
        for rep in range(8):
            w[rep * 16:(rep + 1) * 16, :] = blk
        return w

    per_core_inputs = []
    per_core_rowmaps = []
    for c in cores:
        im = {}
        nvl = len(c["verts"])
        pT = np.zeros((4, NVp), np.float32)
        pT[:3, :nvl] = pos[c["verts"]].T
        pT[3, :] = 1.0

        # entity id list per (k, ci): k=0 -> edge copies; k=1 -> loops; k=2 -> faces
        rowmaps1 = []           # conv1: per class, edge-local -> class row
        rowmap2 = None          # conv2: loop-local -> l2 row
        rowmap3 = None

        for k in range(3):
            subs = sub_schedules(c, k)
            Fk = meta["F"][k]
            rawT = np.zeros((Fk, nblk[k] * 128), np.float32)
            rawT[-1, :] = 1.0
            if k == 1:
                rowmap_out = np.zeros(len(c["loops"]), np.int64)
            elif k == 2:
                rowmap_out = np.zeros(c["hi"] - c["lo"], np.int64)

            for ci, sch in enumerate(subs):
                if k == 0:
                    ent_ids = c["edges"][c["edges_c"][ci]] if ci < len(c["edges_c"]) \
                        else np.zeros(0, np.int64)
                    raws_full = raw_feats[0]
                    rowmap_cls = np.zeros(len(ent_ids), np.int64)
                elif k == 1:
                    ent_ids = c["loops"]
                    raws_full = raw_feats[1]
                else:
                    ent_ids = np.arange(c["lo"], c["hi"])
                    raws_full = raw_feats[2]

                if k == 1:
                    src_rows = meta["e1_rows"][ci] if ci < len(meta["e1_rows"]) else 0
                    idx_arr = im.get(f"gidx1_{ci}")
                    if idx_arr is None and meta["gcols_tot"][1].get(ci, 0) > 0:
                        idx_arr = np.full(
                            (meta["gcols_tot"][1][ci] * 128,), src_rows, np.int64)
                        im[f"gidx1_{ci}"] = idx_arr
                elif k == 2:
                    src_rows = meta["l2_rows"]
                    idx_arr = im.get("gidx2_0")
                    if idx_arr is None:
                        idx_arr = np.full(
                            (meta["gcols_tot"][2][0] * 128,), src_rows, np.int64)
                        im["gidx2_0"] = idx_arr

                for R in BUCKETS:
                    nb = bucket_counts[k].get((ci, R), 0)
                    if nb == 0:
                        continue
                    g = GMAP[R]
                    W = g * R
                    if k == 0:
                        slot_buf = np.zeros((nb // g, 4, W * 128), np.float32)
                    if R in sch:
                        d_all, s_all = sch[R]
                    else:
                        d_all = np.zeros((0, 128), np.int64)
                        s_all = np.zeros((0, 128, R), np.int64)
                    nb_real = d_all.shape[0]
                    cb = col_bases[k][(ci, R)] * 128
                    if nb_real:
                        rows = np.arange(nb_real * 128)
                        dflat = d_all.reshape(-1)
                        if k == 0:
                            ents = c["edges"][c["edges_c"][ci][dflat]]
                            rawT[:-1, cb + rows] = raws_full[ents].T
                        else:
                            rawT[:-1, cb + rows] = raws_full[ent_ids[dflat]].T
                        out_rows = row_bases[k][(ci, R)] + rows
                        if k == 0:
                            rowmap_cls[dflat[::-1]] = out_rows[::-1]
                        else:
                            rowmap_out[dflat[::-1]] = out_rows[::-1]
                    # slots
                    for b in range(nb_real):
                        sl = s_all[b]              # [128, R] local src ids
                        mrow = sl >= 0
                        gi, ci2 = b // g, b % g
                        if k == 0:
                            conv = np.where(mrow, sl, sl[:, :1])
                            for r in range(R):
                                w = r * g + ci2
                                slot_buf[gi, :, w * 128:(w + 1) * 128] = \
                                    pT[:, conv[:, r]]
                        else:
                            conv = np.full(sl.shape, src_rows, np.int64)
                            if k == 1:
                                # slots hold core-local edge ids; the class
                                # rowmap is indexed by class-local position
                                rm = rowmaps1[ci]
                                loc = np.searchsorted(c["edges_c"][ci], sl[mrow])
                                conv[mrow] = rm[loc]
                            else:
                                conv[mrow] = rowmap2[sl[mrow]]
                            gb = gcol_bases[k][(ci, R)]
                            for r in range(R):
                                w = r * g + ci2
                                col = gb + gi * W + w
                                idx_arr[col * 128:(col + 1) * 128] = conv[:, r]
                    if k == 0:
                        im[f"pslot{ci}_{R}"] = slot_buf.astype(NPBF)

                if k == 0:
                    rowmaps1.append(rowmap_cls)

            im[f"rawT{k}"] = rawT.astype(NPBF)
            if k == 1:
                rowmap2 = rowmap_out
            elif k == 2:
                rowmap3 = rowmap_out

        # wrap gather indices to int16 layout
        for key in list(im):
            if key.startswith("gidx"):
                im[key] = wrap16(im[key])

        per_core_inputs.append(im)
        per_core_rowmaps.append([rowmaps1, rowmap2, rowmap3])

    # weights (identical on every core)
    def _lin_w(W_, b_):
        return np.concatenate([np.asarray(W_, np.float32),
                               np.asarray(b_, np.float32)[None]], 0).astype(NPBF)

    wshared = {
        "wv": _lin_w(inputs["Wv"], inputs["bv"]),
        "wx0": _lin_w(inputs["We"], inputs["be"]),
        "wx1": _lin_w(inputs["Wl"], inputs["bl"]),
        "wx2": _lin_w(inputs["Wf"], inputs["bf"]),
    }
    for k, (wn, bn) in enumerate([("Wve", "bve"), ("Wel", "bel"), ("Wlf", "blf")]):
        Wc = np.asarray(inputs[wn], np.float32)
        A, B = Wc[:H], Wc[H:]
        wshared[f"wS{k}"] = (A + B).astype(NPBF)
        wshared[f"wnB{k}"] = (-B).astype(NPBF)
        bc = np.asarray(inputs[bn], np.float32)[None]
        wshared[f"bcr{k}"] = np.tile(bc, (1, 4)).astype(NPBF)
    for im in per_core_inputs:
        im.update(wshared)

    return meta, per_core_inputs, per_core_rowmaps, cores


# ==========================================================================
# Device kernel builder
# ==========================================================================

def _build_kernel(meta, rep=1, use_lrelu=True):
    nblk = meta["nblk"]
    Fs = meta["F"]
    nseg = meta["nseg"]
    bucket_counts = meta["bucket_counts"]
    col_bases = meta["col_bases"]
    row_bases = meta["row_bases"]
    gcol_bases = meta["gcol_bases"]
    gcols_tot = meta["gcols_tot"]

    nc = bacc.Bacc("TRN2", target_bir_lowering=False, debug=False,
                   num_devices=C)

    t_rawT = [nc.dram_tensor(f"rawT{k}", [Fs[k], nblk[k] * 128], bf16,
                             kind="ExternalInput") for k in range(3)]
    t_pslot = {}
    for (ci, R), nb in bucket_counts[0].items():
        g = GMAP[R]
        t_pslot[(ci, R)] = nc.dram_tensor(
            f"pslot{ci}_{R}", [nb // g, 4, g * R * 128], bf16,
            kind="ExternalInput")
    t_gidx = {}
    for k in (1, 2):
        for ci, gc in gcols_tot[k].items():
            if gc > 0:
                t_gidx[(k, ci)] = nc.dram_tensor(
                    f"gidx{k}_{ci}", [128, gc * 8], i16, kind="ExternalInput")
    t_wv = nc.dram_tensor("wv", [4, H], bf16, kind="ExternalInput")
    t_wx = [nc.dram_tensor(f"wx{k}", [Fs[k], H], bf16, kind="ExternalInput")
            for k in range(3)]
    t_wS = [nc.dram_tensor(f"wS{k}", [H, H], bf16, kind="ExternalInput")
            for k in range(3)]
    t_wnB = [nc.dram_tensor(f"wnB{k}", [H, H], bf16, kind="ExternalInput")
             for k in range(3)]
    t_bcr = [nc.dram_tensor(f"bcr{k}", [1, 4 * H], bf16, kind="ExternalInput")
             for k in range(3)]

    # tables: per-class e1 tensors (+1 dummy +BIG row each), l2, f3 output
    t_e1 = [nc.dram_tensor(f"e1_{ci}", [meta["e1_rows"][ci] + 1, H], bf16,
                           kind="Internal") for ci in range(nseg[0])]
    t_l2 = nc.dram_tensor("l2", [meta["l2_rows"] + 1, H], bf16, kind="Internal")
    t_f3 = nc.dram_tensor("f3", [nblk[2] * 128, H], f32, kind="ExternalOutput")

    with tile.TileContext(nc) as tc, ExitStack() as ctx:
        const = ctx.enter_context(tc.tile_pool(name="const", bufs=1))
        pgath = ctx.enter_context(tc.tile_pool(name="pgath", bufs=3))
        pps = ctx.enter_context(tc.tile_pool(name="pps", bufs=3))
        praw = ctx.enter_context(tc.tile_pool(name="praw", bufs=3))
        pxT = ctx.enter_context(tc.tile_pool(name="pxT", bufs=10))
        pmT = ctx.enter_context(tc.tile_pool(name="pmT", bufs=8))
        pbr = ctx.enter_context(tc.tile_pool(name="pbr", bufs=6))
        pmta = ctx.enter_context(tc.tile_pool(name="pmta", bufs=6))
        ptmp = ctx.enter_context(tc.tile_pool(name="ptmp", bufs=10))
        pout = ctx.enter_context(tc.tile_pool(name="pout", bufs=4))
        psA = ctx.enter_context(tc.tile_pool(name="psA", bufs=4, space="PSUM"))
        psT = ctx.enter_context(tc.tile_pool(name="psT", bufs=2, space="PSUM"))
        psO = ctx.enter_context(tc.tile_pool(name="psO", bufs=2, space="PSUM"))

        nc.gpsimd.load_library(library_config.mlp)

        ident = const.tile([128, 128], bf16)
        make_identity(nc, ident[:])
        ones_row = const.tile([1, 128], bf16)
        nc.vector.memset(ones_row[:], 1.0)

        # resident weights
        wv_sb = const.tile([4, H], bf16)
        nc.scalar.dma_start(out=wv_sb[:], in_=t_wv.ap()[:, :])
        wx_sb = []
        for k in range(3):
            t = const.tile([Fs[k], H], bf16, tag=f"wx{k}")
            nc.scalar.dma_start(out=t[:], in_=t_wx[k].ap()[:, :])
            wx_sb.append(t)
        wS_sb, wnB_sb = [], []
        for k in range(3):
            cs_, cb_ = [], []
            for j in range(2):
                t = const.tile([128, H], bf16, tag=f"wS{k}_{j}")
                nc.scalar.dma_start(out=t[:], in_=t_wS[k].ap()[j * 128:(j + 1) * 128, :])
                cs_.append(t)
                t = const.tile([128, H], bf16, tag=f"wnB{k}_{j}")
                nc.scalar.dma_start(out=t[:], in_=t_wnB[k].ap()[j * 128:(j + 1) * 128, :])
                cb_.append(t)
            wS_sb.append(cs_)
            wnB_sb.append(cb_)
        bcr_sb = []
        for k in range(3):
            b1 = const.tile([1, 4 * H], bf16, tag=f"bcr{k}")
            nc.scalar.dma_start(out=b1[:], in_=t_bcr[k].ap()[:, :])
            bcr_sb.append(b1)

        # dummy (+BIG) rows for the gather tables
        bigt = const.tile([1, H], bf16)
        nc.vector.memset(bigt[:], float(BIG))
        for ci in range(nseg[0]):
            nc.sync.dma_start(out=t_e1[ci].ap()[meta["e1_rows"][ci]:, :],
                              in_=bigt[:])
        nc.sync.dma_start(out=t_l2.ap()[meta["l2_rows"]:, :], in_=bigt[:])

        # resident gather indices (int16 wrapped layout), one tile per (k, ci)
        gidx_sb = {}
        for (k, ci), t in t_gidx.items():
            it = const.tile(list(t.shape), i16, tag=f"gidx{k}_{ci}")
            nc.sync.dma_start(out=it[:], in_=t.ap()[:, :])
            gidx_sb[(k, ci)] = it

        def leaky(out_ap, in_ap, ncols):
            """leaky(z) = max(z, 0.01 z); native Lrelu on Act when enabled."""
            if use_lrelu:
                nc.scalar.activation(out=out_ap, in_=in_ap,
                                     func=mybir.ActivationFunctionType.Lrelu,
                                     alpha=0.01)
            else:
                tt = ptmp.tile([128, 2 * H], bf16, tag="ttl")
                nc.scalar.mul(out=tt[:, :ncols], in_=in_ap, mul=0.01)
                nc.vector.tensor_tensor(out=out_ap, in0=in_ap,
                                        in1=tt[:, :ncols], op=ALU.max)

        tables = [None, t_e1, [t_l2]]
        outs = [t_e1, [t_l2], [t_f3]]

        def do_group(k, ci, R, gi, pst, rawl, gts):
            """One g-group: 128g dsts of conv k, class ci, bucket R.
            gts = SBUF slice [128, W*H] of gathered source rows (k>0)."""
            g = GMAP[R]
            W = g * R
            wx = wx_sb[k]
            base = row_bases[k][(ci, R)] + gi * g * 128
            out_t = outs[k][ci if k == 0 else 0]

            if k == 0:
                # transposed min-accumulate: zT = wv_chunk.T @ posSlot lands
                # [h-dims, slots] directly, so no PE transposes needed.
                mx = []
                for h2 in range(2):
                    if R == 1:
                        # single source: leaky straight off PSUM in one Act op
                        zTr = psA.tile([128, 512], f32, tag="psA")
                        nc.tensor.matmul(
                            out=zTr[:, :g * 128],
                            lhsT=wv_sb[:, h2 * 128:(h2 + 1) * 128],
                            rhs=pst[:, :g * 128],
                            start=True, stop=True)
                        mm = pmT.tile([128, g * 128], bf16, tag="mT")
                        leaky(mm[:], zTr[:, :g * 128], g * 128)
                        mx.append(mm)
                        continue
                    mta = pmta.tile([128, 512], bf16, tag="mTacc")
                    for r in range(R):
                        zTr = psA.tile([128, 512], f32, tag="psA")
                        nc.tensor.matmul(
                            out=zTr[:, :g * 128],
                            lhsT=wv_sb[:, h2 * 128:(h2 + 1) * 128],
                            rhs=pst[:, r * g * 128:(r + 1) * g * 128],
                            start=True, stop=True)
                        if r == 0:
                            # DVE can't read two PSUM operands; Act copy
                            nc.scalar.copy(out=mta[:, :g * 128],
                                           in_=zTr[:, :g * 128])
                        else:
                            nc.vector.tensor_tensor(out=mta[:, :g * 128],
                                                    in0=mta[:, :g * 128],
                                                    in1=zTr[:, :g * 128],
                                                    op=ALU.min)
                    # leaky on DVE (SBUF bf16 so the max runs in 2x mode)
                    ttm = pbr.tile([128, g * 128], bf16, tag="ttm")
                    nc.vector.tensor_scalar(out=ttm[:], in0=mta[:, :g * 128],
                                            scalar1=0.01, scalar2=None,
                                            op0=ALU.mult)
                    mm = pmT.tile([128, g * 128], bf16, tag="mT")
                    nc.vector.tensor_tensor(out=mm[:], in0=mta[:, :g * 128],
                                            in1=ttm[:], op=ALU.max)
                    mx.append(mm)
            else:
                # min fold over rounds (col w = r*g + c keeps block assoc)
                gt = gts
                s = R
                while s > 1:
                    h = s // 2
                    nc.vector.tensor_tensor(
                        out=gt[:, : h * g * H], in0=gt[:, : h * g * H],
                        in1=gt[:, h * g * H: 2 * h * g * H], op=ALU.min)
                    if s % 2:
                        nc.vector.tensor_tensor(
                            out=gt[:, : g * H], in0=gt[:, : g * H],
                            in1=gt[:, (s - 1) * g * H: s * g * H],
                            op=ALU.min)
                    s = h

            # x_dst transposed halves: xT = leaky(wx.T @ rawl)
            xT = []
            for h2 in range(2):
                zT = psA.tile([128, g * 128], f32, tag="psA")
                nc.tensor.matmul(out=zT[:],
                                 lhsT=wx[:, h2 * 128:(h2 + 1) * 128],
                                 rhs=rawl[:], start=True, stop=True)
                xs = pxT.tile([128, g * 128], bf16, tag="xT")
                leaky(xs[:], zT[:], g * 128)
                xT.append(xs)

            if k > 0:
                # mT via PE transpose of min columns (both halves one tile)
                zmT = psT.tile([128, 4 * H], bf16, tag="psT")
                for h2 in range(2):
                    for ci2 in range(g):
                        base_m = ci2 * H
                        nc.tensor.transpose(
                            out=zmT[:, (h2 * g + ci2) * 128:(h2 * g + ci2 + 1) * 128],
                            in_=gt[:, base_m + h2 * 128: base_m + (h2 + 1) * 128],
                            identity=ident[:])
                mmb = pmT.tile([128, 2 * g * 128], bf16, tag="mT")
                nc.vector.tensor_scalar(out=mmb[:], in0=zmT[:, :2 * g * 128],
                                        scalar1=1.0, scalar2=None,
                                        op0=ALU.mult)
                mx = [mmb[:, :g * 128], mmb[:, g * 128:2 * g * 128]]

            # x row-major via bf16 PE transpose of xT (independent of zo)
            px = psT.tile([128, 4 * H], bf16, tag="psT")
            for ci2 in range(g):
                for h2 in range(2):
                    nc.tensor.transpose(
                        out=px[:, ci2 * H + h2 * 128: ci2 * H + (h2 + 1) * 128],
                        in_=xT[h2][:, ci2 * 128:(ci2 + 1) * 128],
                        identity=ident[:])

            out_sb = pout.tile([128, g * H], f32 if k == 2 else bf16, tag="outC")
            for p2 in range(0, g, 2):
                pw = min(2, g - p2)
                # zo = x@S + m@(-B) + bc
                zo = psO.tile([128, 2 * H], f32, tag="psO")
                for c2 in range(pw):
                    ci2 = p2 + c2
                    cs = slice(ci2 * 128, (ci2 + 1) * 128)
                    zr = zo[:, c2 * H:(c2 + 1) * H]
                    nc.tensor.matmul(out=zr, lhsT=ones_row[:],
                                     rhs=bcr_sb[k][:, :H], start=True, stop=False)
                    nc.tensor.matmul(out=zr, lhsT=xT[0][:, cs], rhs=wS_sb[k][0][:],
                                     start=False, stop=False)
                    nc.tensor.matmul(out=zr, lhsT=xT[1][:, cs], rhs=wS_sb[k][1][:],
                                     start=False, stop=False)
                    nc.tensor.matmul(out=zr, lhsT=mx[0][:, cs], rhs=wnB_sb[k][0][:],
                                     start=False, stop=False)
                    nc.tensor.matmul(out=zr, lhsT=mx[1][:, cs], rhs=wnB_sb[k][1][:],
                                     start=False, stop=True)

                # out = x + leaky(zo)
                lk = ptmp.tile([128, 2 * H], bf16, tag="lk")
                leaky(lk[:, :pw * H], zo[:, :pw * H], pw * H)
                nc.vector.tensor_tensor(out=out_sb[:, p2 * H:(p2 + pw) * H],
                                        in0=px[:, p2 * H:(p2 + pw) * H],
                                        in1=lk[:, :pw * H],
                                        op=ALU.add)
            nc.sync.dma_start(
                out=out_t.ap()[base:base + g * 128, :]
                    .rearrange("(c p) d -> p c d", p=128),
                in_=out_sb[:].rearrange("p (c d) -> p c d", d=H))

        import os as _os
        _kset = [int(x) for x in _os.environ.get("BREP_KSET", "012")]
        for _rep in range(rep):
          for k in _kset:
            for ci in range(nseg[k]):
              for R in BUCKETS:
                  nb = bucket_counts[k].get((ci, R), 0)
                  if nb == 0:
                      continue
                  g = GMAP[R]
                  W = g * R
                  ngr = nb // g
                  cb = col_bases[k][(ci, R)]
                  # chunked staging loads: pslot (conv1) / rawT columns
                  ck_ps = max(1, 4096 // (W * 128))
                  ck_rw = max(1, 4096 // (g * 128))
                  ck = min(ck_ps, ck_rw) if k == 0 else ck_rw
                  # gather sub-chunk (groups per dma_gather)
                  gcg = max(1, GCOLS // W)
                  for g0 in range(0, ngr, ck):
                      n_in = min(ck, ngr - g0)
                      if k == 0:
                          pch = pps.tile([4, ck * W * 128], bf16, tag="pslot")
                          nc.sync.dma_start(
                              out=pch[:, :n_in * W * 128]
                                  .rearrange("p (n w) -> p n w", w=W * 128),
                              in_=t_pslot[(ci, R)].ap()[g0:g0 + n_in]
                                  .rearrange("n p w -> p n w"))
                      rch = praw.tile([Fs[k], ck * g * 128], bf16, tag="rawC")
                      ccb = (cb + g0 * g) * 128
                      nc.sync.dma_start(
                          out=rch[:, :n_in * g * 128],
                          in_=t_rawT[k].ap()[:, ccb:ccb + n_in * g * 128])
                      if k > 0:
                          src_t = tables[k][ci if k == 1 else 0]
                          gcb = gcol_bases[k][(ci, R)]
                          gchunks = {}
                          for gg in range(g0, g0 + n_in, gcg):
                              n_g = min(gcg, g0 + n_in - gg)
                              cols = n_g * W
                              gt = pgath.tile([128, 32 * H], bf16, tag="g")
                              c0 = gcb + gg * W
                              # single_packet=False: packed-stream mode wedges
                              # the device beyond ~64 descriptors per ring
                              nc.gpsimd.dma_gather(
                                  gt[:, :cols * H]
                                      .rearrange("p (g e) -> p g e", e=H),
                                  src_t.ap()[:, :],
                                  gidx_sb[(k, ci)][:, c0 * 8:(c0 + cols) * 8],
                                  cols * 128, cols * 128, H,
                                  single_packet=False)
                              gchunks[gg] = gt
                      for gi in range(g0, g0 + n_in):
                          off = gi - g0
                          pst = (pch[:, off * W * 128:(off + 1) * W * 128]
                                 if k == 0 else None)
                          rawl = rch[:, off * g * 128:(off + 1) * g * 128]
                          gts = None
                          if k > 0:
                              gg = g0 + ((gi - g0) // gcg) * gcg
                              goff = gi - gg
                              gts = gchunks[gg][:, goff * W * H:(goff + 1) * W * H]
                          do_group(k, ci, R, gi, pst, rawl, gts)

    nc.compile()
    return nc


# ==========================================================================
# Entry point
# ==========================================================================

def kernel(**inputs):
    import os
    meta, per_core_inputs, per_core_rowmaps, cores = _host_prep(inputs)
    nc = _build_kernel(meta,
                       use_lrelu=not os.environ.get("BREP_NO_LRELU"))

    in_maps = [dict(im) for im in per_core_inputs]

    if os.environ.get("BREP_SIM"):
        from concourse.bass_interp import CoreSim
        nc_sim = _build_kernel(meta, use_lrelu=False)  # interp can't exec Lrelu
        results = []
        for ci in range(int(os.environ.get("BREP_SIM_CORES", C))):
            sim = CoreSim(nc_sim, trace=False)
            for name, arr in in_maps[ci].items():
                sim.tensor(name)[:] = arr
            sim.simulate()
            results.append({"f3": np.array(sim.tensor("f3"))})
    else:
        res = run_bass_kernel_spmd(nc, in_maps, core_ids=list(range(C)))
        results = res.results

    NF = inputs["face_surfaces"].shape[0]
    out = np.empty((NF, H), np.float32)
    for ci, (r, c) in enumerate(zip(results, cores)):
        f3 = r["f3"]
        rm = per_core_rowmaps[ci][2]          # local face -> table row
        out[c["lo"]:c["hi"]] = f3[rm]
    return out


# revision 12
# speedup vs baseline: 3.0881x; 3.0881x over previous
"""Trainium2 Bass kernel for nn_BRepFaceEncoder (gnn_message_passing).

Sharding: the 60000 faces are split contiguously across 8 NeuronCores. Each
core back-chains the halo it needs (faces -> loops -> edges -> vertices) and
runs the whole pipeline locally - no collectives.

Math identities used:
  segment_max_d(x_dst[d] - x_src[s]) == x_dst[d] - segment_min_s(x_src[s])
  min(leaky(z)) == leaky(min(z))   (monotone; exact - conv1 only)
  concat([x, x - m]) @ Wc == x @ (A + B) + m @ (-B)   (A=Wc[:H], B=Wc[H:])

All compute in bf16 (PE matmul 1 cyc/row vs 4 for fp32; DVE 2x on 16-bit),
fp32 PSUM accumulation. conv1 needs no gather: raw vertex positions are
host-staged into per-round slot order and min-accumulated in pre-activation
space. conv2/3 gather previous-layer rows with gpsimd dma_gather ops
(HW-verified exact; multi-index indirect_dma_start is NOT - the firmware
reads only offset[p,0] and strides rows contiguously). dma_gather indices
are int16, so conv1's output table is split into <=32766-row "classes":
conv2's blocks are greedily partitioned into contiguous ranges whose unique
sources fit a class; edges used by several classes are duplicated into each
(few % extra conv1 compute). Each class is its own DRAM tensor, which also
gives the tile scheduler class-granular write->gather dependencies.

Destinations are degree-sorted into 128-row blocks bucketed by round count R.
Leaky runs as a single native Lrelu op on the Act engine (alpha=0.01,
HW-verified); conv1's min accumulates directly in transposed space via
wv-chunk-as-lhsT matmuls, so no PE transposes are needed there (round 0
needs an Act copy off PSUM - DVE cannot read two PSUM operands).

dma_gather notes (HW-verified): single_packet=True wedges the device beyond
~64 descriptors per ring - always pass single_packet=False for big gathers.
idx tiles may be sliced at arbitrary column offsets; Internal-tensor sources
and elem_step row strides work.
"""

import sys
from contextlib import ExitStack

import numpy as np
import ml_dtypes

if "/opt/trn_rl_repo" not in sys.path:
    sys.path.insert(0, "/opt/trn_rl_repo")

import concourse.bass as bass            # noqa: E402
import concourse.tile as tile            # noqa: E402
from concourse import bacc, mybir, library_config  # noqa: E402
from concourse.bass_utils import run_bass_kernel_spmd  # noqa: E402
from concourse.masks import make_identity              # noqa: E402

f32 = mybir.dt.float32
bf16 = mybir.dt.bfloat16
i16 = mybir.dt.int16
ALU = mybir.AluOpType
NPBF = ml_dtypes.bfloat16

H = 256
C = 8
BIG = np.float32(512.0)
BUCKETS = (1, 2, 3, 4, 5, 6, 7, 8, 9, 10, 12, 14, 16, 20, 24, 32)
GMAP = {1: 4, 2: 4, 3: 4, 4: 4, 5: 2, 6: 2, 7: 2, 8: 2, 9: 1, 10: 1,
        12: 1, 14: 1, 16: 1, 20: 1, 24: 1, 32: 1}
CLASS_CAP = 30500          # max unique sources per e1 class (idx fits int16)
GCOLS = 16                 # target grid columns per dma_gather op


# ==========================================================================
# Host-side schedule construction
# ==========================================================================

def _build_conv_schedule(dloc, sloc, n_dst):
    """Blocks of 128 degree-sorted dsts, bucketed by round count R."""
    counts = np.bincount(dloc, minlength=n_dst)
    order_p = np.argsort(dloc, kind="stable")
    srcs_sorted = sloc[order_p]
    starts = np.zeros(n_dst + 1, dtype=np.int64)
    np.cumsum(counts, out=starts[1:])

    perm = np.argsort(-counts, kind="stable")
    n_blk = (n_dst + 127) // 128
    pad = n_blk * 128 - n_dst
    perm_padded = np.concatenate([perm, np.full(pad, perm[-1] if n_dst else 0,
                                                dtype=perm.dtype)])
    deg_padded = counts[perm_padded]
    deg_padded[n_dst:] = 0

    bucket_blocks = {}
    for b in range(n_blk):
        dsts = perm_padded[b * 128:(b + 1) * 128]
        degs = deg_padded[b * 128:(b + 1) * 128]
        mx = int(degs[0])
        R = next(r for r in BUCKETS if r >= max(mx, 1))
        slots = np.full((128, R), -1, dtype=np.int64)
        base = starts[dsts]
        for r in range(R):
            have = degs > r
            if have.any():
                slots[have, r] = srcs_sorted[base[have] + r]
        d, s = bucket_blocks.setdefault(R, ([], []))
        d.append(dsts)
        s.append(slots)
    return {R: (np.stack(d), np.stack(s)) for R, (d, s) in bucket_blocks.items()}


def _split_classes(sch, cap):
    """Greedy partition of blocks (group granularity, layout order) into
    classes whose unique source sets stay <= cap. Returns a list of
    per-class schedules ({R: (d, s)}) and per-class sorted unique sources."""
    classes = []
    cur = {}
    cur_set = set()

    def flush():
        nonlocal cur, cur_set
        if cur:
            sch_c = {R: (np.stack(d), np.stack(s)) for R, (d, s) in cur.items()}
            srcs = np.array(sorted(cur_set), dtype=np.int64)
            classes.append((sch_c, srcs))
        cur, cur_set = {}, set()

    for R in BUCKETS:
        if R not in sch:
            continue
        d_all, s_all = sch[R]
        g = GMAP[R]
        nb = d_all.shape[0]
        for g0 in range(0, nb, g):
            blk_d = d_all[g0:g0 + g]
            blk_s = s_all[g0:g0 + g]
            srcs = set(blk_s[blk_s >= 0].tolist())
            if cur_set and len(cur_set | srcs) > cap:
                flush()
            dd, ss = cur.setdefault(R, ([], []))
            for b in range(blk_d.shape[0]):
                dd.append(blk_d[b])
                ss.append(blk_s[b])
            cur_set |= srcs
    flush()
    return classes


def _host_prep(inputs):
    e2v = np.asarray(inputs["edge_to_vertex"])
    l2e = np.asarray(inputs["loop_to_edge"])
    f2l = np.asarray(inputs["face_to_loop"])

    NV = inputs["vertex_positions"].shape[0]
    NE = inputs["edge_curves"].shape[0]
    NL = inputs["loop_types"].shape[0]
    NF = inputs["face_surfaces"].shape[0]

    pos = np.asarray(inputs["vertex_positions"], np.float32)
    raw_feats = [
        np.concatenate([np.asarray(inputs["edge_curves"], np.float32),
                        np.asarray(inputs["edge_curve_parameters"], np.float32),
                        np.asarray(inputs["edge_curve_flipped"], np.float32)[:, None]], axis=1),
        np.asarray(inputs["loop_types"], np.float32),
        np.concatenate([np.asarray(inputs["face_surfaces"], np.float32),
                        np.asarray(inputs["face_surface_parameters"], np.float32),
                        np.asarray(inputs["face_surface_flipped"], np.float32)[:, None]], axis=1),
    ]

    cores = []
    for i in range(C):
        lo, hi = i * NF // C, (i + 1) * NF // C
        mask = np.zeros(NF, bool); mask[lo:hi] = True
        m3 = mask[f2l[0]]
        d3, s3 = f2l[0][m3] - lo, f2l[1][m3]
        loops_i = np.unique(s3)
        mask = np.zeros(NL, bool); mask[loops_i] = True
        m2 = mask[l2e[0]]
        d2, s2 = l2e[0][m2], l2e[1][m2]
        edges_i = np.unique(s2)
        mask = np.zeros(NE, bool); mask[edges_i] = True
        m1 = mask[e2v[0]]
        d1, s1 = e2v[0][m1], e2v[1][m1]
        verts_i = np.unique(s1)

        d1l = np.searchsorted(edges_i, d1)
        s1l = np.searchsorted(verts_i, s1)
        d2l = np.searchsorted(loops_i, d2)
        s2l = np.searchsorted(edges_i, s2)
        s3l = np.searchsorted(loops_i, s3)

        sch2_full = _build_conv_schedule(d2l, s2l, len(loops_i))
        sch3 = _build_conv_schedule(d3, s3l, hi - lo)

        # class split of conv2 blocks by unique edge sources
        cls2 = _split_classes(sch2_full, CLASS_CAP)

        # conv1 sub-schedule per class (over that class's edge copies)
        sch1_c, edges_c = [], []
        for _, srcs in cls2:
            mark = np.zeros(len(edges_i), bool)
            mark[srcs] = True
            selp = mark[d1l]
            d1c = np.searchsorted(srcs, d1l[selp])
            sch1_c.append(_build_conv_schedule(d1c, s1l[selp], len(srcs)))
            edges_c.append(srcs)

        cores.append(dict(lo=lo, hi=hi, loops=loops_i, edges=edges_i,
                          verts=verts_i, sch1_c=sch1_c, edges_c=edges_c,
                          cls2=[s for s, _ in cls2], sch3=sch3))

    NSEG1 = max(len(c["sch1_c"]) for c in cores)
    NVp = ((max(len(c["verts"]) for c in cores) + 511) // 512) * 512

    # global padded block counts per (k, class, R); k=0 classes = NSEG1
    def sub_schedules(c, k):
        if k == 0:
            return c["sch1_c"] + [{}] * (NSEG1 - len(c["sch1_c"]))
        if k == 1:
            return c["cls2"] + [{}] * (NSEG1 - len(c["cls2"]))
        return [c["sch3"]]

    nseg = [NSEG1, NSEG1, 1]
    bucket_counts = [{}, {}, {}]
    for k in range(3):
        for c in cores:
            for ci, sch in enumerate(sub_schedules(c, k)):
                for R, (d, s) in sch.items():
                    g = GMAP[R]
                    n = -(-d.shape[0] // g) * g
                    key = (ci, R)
                    bucket_counts[k][key] = max(bucket_counts[k].get(key, 0), n)
    nblk = [sum(bucket_counts[k].values()) for k in range(3)]

    # e1 class tensor sizes (rows, +1 dummy each); l2 single table
    e1_rows = []
    for ci in range(NSEG1):
        n = sum(nb for (cc, R), nb in bucket_counts[0].items() if cc == ci) * 128
        assert n + 1 <= 32767, f"e1 class {ci} too big: {n}"
        e1_rows.append(n)
    l2_rows = nblk[1] * 128
    assert l2_rows + 1 <= 32767, f"l2 too big: {l2_rows}"

    meta = dict(NVp=NVp, bucket_counts=bucket_counts, nblk=nblk, nseg=nseg,
                e1_rows=e1_rows, l2_rows=l2_rows, F=[16, 11, 18])

    # layout bases: global column base per (k, ci, R); local row base for e1
    col_bases = [{}, {}, {}]     # rawT column base (in blocks)
    row_bases = [{}, {}, {}]     # output row base (local to class tensor for k=0)
    for k in range(3):
        cb = 0
        rb_cls = {}
        for ci in range(nseg[k]):
            for R in BUCKETS:
                nb = bucket_counts[k].get((ci, R), 0)
                if nb == 0:
                    continue
                col_bases[k][(ci, R)] = cb
                if k == 0:
                    row_bases[k][(ci, R)] = rb_cls.get(ci, 0)
                    rb_cls[ci] = rb_cls.get(ci, 0) + nb * 128
                else:
                    row_bases[k][(ci, R)] = cb * 128
                cb += nb
    meta["col_bases"] = col_bases
    meta["row_bases"] = row_bases

    # grid-column bases per (k, ci, R) for gather index arrays (k=1,2),
    # local to the (k, ci) index tensor
    gcol_bases = [None, {}, {}]
    gcols_tot = [None, {}, {}]
    for k in (1, 2):
        for ci in range(nseg[k]):
            gc = 0
            for R in BUCKETS:
                nb = bucket_counts[k].get((ci, R), 0)
                if nb == 0:
                    continue
                gcol_bases[k][(ci, R)] = gc
                gc += nb * R
            gcols_tot[k][ci] = gc
    meta["gcol_bases"] = gcol_bases
    meta["gcols_tot"] = gcols_tot

    def wrap16(flat):
        """[n*128] int array -> dma_gather wrapped [128, n*8] int16 layout."""
        n = flat.shape[0] // 16
        w = np.empty((128, n), np.int16)
        blk = flat.reshape(n, 16).T.astype(np.int16)   # [16, n]
        for rep in range(8):
            w[rep * 16:(rep + 1) * 16, :] = blk
        return w

    per_core_inputs = []
    per_core_rowmaps = []
    for c in cores:
        im = {}
        nvl = len(c["verts"])
        pT = np.zeros((4, NVp), np.float32)
        pT[:3, :nvl] = pos[c["verts"]].T
        pT[3, :] = 1.0

        # entity id list per (k, ci): k=0 -> edge copies; k=1 -> loops; k=2 -> faces
        rowmaps1 = []           # conv1: per class, edge-local -> class row
        rowmap2 = None          # conv2: loop-local -> l2 row
        rowmap3 = None

        for k in range(3):
            subs = sub_schedules(c, k)
            Fk = meta["F"][k]
            rawT = np.zeros((Fk, nblk[k] * 128), np.float32)
            rawT[-1, :] = 1.0
            if k == 1:
                rowmap_out = np.zeros(len(c["loops"]), np.int64)
            elif k == 2:
                rowmap_out = np.zeros(c["hi"] - c["lo"], np.int64)

            for ci, sch in enumerate(subs):
                if k == 0:
                    ent_ids = c["edges"][c["edges_c"][ci]] if ci < len(c["edges_c"]) \
                        else np.zeros(0, np.int64)
                    raws_full = raw_feats[0]
                    rowmap_cls = np.zeros(len(ent_ids), np.int64)
                elif k == 1:
                    ent_ids = c["loops"]
                    raws_full = raw_feats[1]
                else:
                    ent_ids = np.arange(c["lo"], c["hi"])
                    raws_full = raw_feats[2]

                if k == 1:
                    src_rows = meta["e1_rows"][ci] if ci < len(meta["e1_rows"]) else 0
                    idx_arr = im.get(f"gidx1_{ci}")
                    if idx_arr is None and meta["gcols_tot"][1].get(ci, 0) > 0:
                        idx_arr = np.full(
                            (meta["gcols_tot"][1][ci] * 128,), src_rows, np.int64)
                        im[f"gidx1_{ci}"] = idx_arr
                elif k == 2:
                    src_rows = meta["l2_rows"]
                    idx_arr = im.get("gidx2_0")
                    if idx_arr is None:
                        idx_arr = np.full(
                            (meta["gcols_tot"][2][0] * 128,), src_rows, np.int64)
                        im["gidx2_0"] = idx_arr

                for R in BUCKETS:
                    nb = bucket_counts[k].get((ci, R), 0)
                    if nb == 0:
                        continue
                    g = GMAP[R]
                    W = g * R
                    if k == 0:
                        slot_buf = np.zeros((nb // g, 4, W * 128), np.float32)
                    if R in sch:
                        d_all, s_all = sch[R]
                    else:
                        d_all = np.zeros((0, 128), np.int64)
                        s_all = np.zeros((0, 128, R), np.int64)
                    nb_real = d_all.shape[0]
                    cb = col_bases[k][(ci, R)] * 128
                    if nb_real:
                        rows = np.arange(nb_real * 128)
                        dflat = d_all.reshape(-1)
                        if k == 0:
                            ents = c["edges"][c["edges_c"][ci][dflat]]
                            rawT[:-1, cb + rows] = raws_full[ents].T
                        else:
                            rawT[:-1, cb + rows] = raws_full[ent_ids[dflat]].T
                        out_rows = row_bases[k][(ci, R)] + rows
                        if k == 0:
                            rowmap_cls[dflat[::-1]] = out_rows[::-1]
                        else:
                            rowmap_out[dflat[::-1]] = out_rows[::-1]
                    # slots
                    for b in range(nb_real):
                        sl = s_all[b]              # [128, R] local src ids
                        mrow = sl >= 0
                        gi, ci2 = b // g, b % g
                        if k == 0:
                            conv = np.where(mrow, sl, sl[:, :1])
                            for r in range(R):
                                w = r * g + ci2
                                slot_buf[gi, :, w * 128:(w + 1) * 128] = \
                                    pT[:, conv[:, r]]
                        else:
                            conv = np.full(sl.shape, src_rows, np.int64)
                            if k == 1:
                                # slots hold core-local edge ids; the class
                                # rowmap is indexed by class-local position
                                rm = rowmaps1[ci]
                                loc = np.searchsorted(c["edges_c"][ci], sl[mrow])
                                conv[mrow] = rm[loc]
                            else:
                                conv[mrow] = rowmap2[sl[mrow]]
                            gb = gcol_bases[k][(ci, R)]
                            for r in range(R):
                                w = r * g + ci2
                                col = gb + gi * W + w
                                idx_arr[col * 128:(col + 1) * 128] = conv[:, r]
                    if k == 0:
                        im[f"pslot{ci}_{R}"] = slot_buf.astype(NPBF)

                if k == 0:
                    rowmaps1.append(rowmap_cls)

            im[f"rawT{k}"] = rawT.astype(NPBF)
            if k == 1:
                rowmap2 = rowmap_out
            elif k == 2:
                rowmap3 = rowmap_out

        # wrap gather indices to int16 layout
        for key in list(im):
            if key.startswith("gidx"):
                im[key] = wrap16(im[key])

        per_core_inputs.append(im)
        per_core_rowmaps.append([rowmaps1, rowmap2, rowmap3])

    # weights (identical on every core)
    def _lin_w(W_, b_):
        return np.concatenate([np.asarray(W_, np.float32),
                               np.asarray(b_, np.float32)[None]], 0).astype(NPBF)

    wshared = {
        "wv": _lin_w(inputs["Wv"], inputs["bv"]),
        "wx0": _lin_w(inputs["We"], inputs["be"]),
        "wx1": _lin_w(inputs["Wl"], inputs["bl"]),
        "wx2": _lin_w(inputs["Wf"], inputs["bf"]),
    }
    for k, (wn, bn) in enumerate([("Wve", "bve"), ("Wel", "bel"), ("Wlf", "blf")]):
        Wc = np.asarray(inputs[wn], np.float32)
        A, B = Wc[:H], Wc[H:]
        wshared[f"wS{k}"] = (A + B).astype(NPBF)
        wshared[f"wnB{k}"] = (-B).astype(NPBF)
        bc = np.asarray(inputs[bn], np.float32)[None]
        wshared[f"bcr{k}"] = np.tile(bc, (1, 4)).astype(NPBF)
    for im in per_core_inputs:
        im.update(wshared)

    return meta, per_core_inputs, per_core_rowmaps, cores


# ==========================================================================
# Device kernel builder
# ==========================================================================

def _build_kernel(meta, rep=1, use_lrelu=True):
    import os as _os
    gp_fold = bool(int(_os.environ.get("BREP_GP_FOLD", "0")))
    gp_leaky = bool(int(_os.environ.get("BREP_GP_LEAKY", "0")))
    nblk = meta["nblk"]
    Fs = meta["F"]
    nseg = meta["nseg"]
    bucket_counts = meta["bucket_counts"]
    col_bases = meta["col_bases"]
    row_bases = meta["row_bases"]
    gcol_bases = meta["gcol_bases"]
    gcols_tot = meta["gcols_tot"]

    nc = bacc.Bacc("TRN2", target_bir_lowering=False, debug=False,
                   num_devices=C)

    t_rawT = [nc.dram_tensor(f"rawT{k}", [Fs[k], nblk[k] * 128], bf16,
                             kind="ExternalInput") for k in range(3)]
    t_pslot = {}
    for (ci, R), nb in bucket_counts[0].items():
        g = GMAP[R]
        t_pslot[(ci, R)] = nc.dram_tensor(
            f"pslot{ci}_{R}", [nb // g, 4, g * R * 128], bf16,
            kind="ExternalInput")
    t_gidx = {}
    for k in (1, 2):
        for ci, gc in gcols_tot[k].items():
            if gc > 0:
                t_gidx[(k, ci)] = nc.dram_tensor(
                    f"gidx{k}_{ci}", [128, gc * 8], i16, kind="ExternalInput")
    t_wv = nc.dram_tensor("wv", [4, H], bf16, kind="ExternalInput")
    t_wx = [nc.dram_tensor(f"wx{k}", [Fs[k], H], bf16, kind="ExternalInput")
            for k in range(3)]
    t_wS = [nc.dram_tensor(f"wS{k}", [H, H], bf16, kind="ExternalInput")
            for k in range(3)]
    t_wnB = [nc.dram_tensor(f"wnB{k}", [H, H], bf16, kind="ExternalInput")
             for k in range(3)]
    t_bcr = [nc.dram_tensor(f"bcr{k}", [1, 4 * H], bf16, kind="ExternalInput")
             for k in range(3)]

    # tables: per-class e1 tensors (+1 dummy +BIG row each), l2, f3 output
    t_e1 = [nc.dram_tensor(f"e1_{ci}", [meta["e1_rows"][ci] + 1, H], bf16,
                           kind="Internal") for ci in range(nseg[0])]
    t_l2 = nc.dram_tensor("l2", [meta["l2_rows"] + 1, H], bf16, kind="Internal")
    t_f3 = nc.dram_tensor("f3", [nblk[2] * 128, H], f32, kind="ExternalOutput")

    with tile.TileContext(nc) as tc, ExitStack() as ctx:
        const = ctx.enter_context(tc.tile_pool(name="const", bufs=1))
        pgath = ctx.enter_context(tc.tile_pool(name="pgath", bufs=3))
        pps = ctx.enter_context(tc.tile_pool(name="pps", bufs=3))
        praw = ctx.enter_context(tc.tile_pool(name="praw", bufs=3))
        pxT = ctx.enter_context(tc.tile_pool(name="pxT", bufs=10))
        pmT = ctx.enter_context(tc.tile_pool(name="pmT", bufs=8))
        pbr = ctx.enter_context(tc.tile_pool(name="pbr", bufs=6))
        pmta = ctx.enter_context(tc.tile_pool(name="pmta", bufs=6))
        ptmp = ctx.enter_context(tc.tile_pool(name="ptmp", bufs=10))
        pout = ctx.enter_context(tc.tile_pool(name="pout", bufs=4))
        psA = ctx.enter_context(tc.tile_pool(name="psA", bufs=4, space="PSUM"))
        psT = ctx.enter_context(tc.tile_pool(name="psT", bufs=2, space="PSUM"))
        psO = ctx.enter_context(tc.tile_pool(name="psO", bufs=2, space="PSUM"))

        nc.gpsimd.load_library(library_config.mlp)

        ident = const.tile([128, 128], bf16)
        make_identity(nc, ident[:])
        ones_row = const.tile([1, 128], bf16)
        nc.vector.memset(ones_row[:], 1.0)

        # resident weights
        wv_sb = const.tile([4, H], bf16)
        nc.scalar.dma_start(out=wv_sb[:], in_=t_wv.ap()[:, :])
        wx_sb = []
        for k in range(3):
            t = const.tile([Fs[k], H], bf16, tag=f"wx{k}")
            nc.scalar.dma_start(out=t[:], in_=t_wx[k].ap()[:, :])
            wx_sb.append(t)
        wS_sb, wnB_sb = [], []
        for k in range(3):
            cs_, cb_ = [], []
            for j in range(2):
                t = const.tile([128, H], bf16, tag=f"wS{k}_{j}")
                nc.scalar.dma_start(out=t[:], in_=t_wS[k].ap()[j * 128:(j + 1) * 128, :])
                cs_.append(t)
                t = const.tile([128, H], bf16, tag=f"wnB{k}_{j}")
                nc.scalar.dma_start(out=t[:], in_=t_wnB[k].ap()[j * 128:(j + 1) * 128, :])
                cb_.append(t)
            wS_sb.append(cs_)
            wnB_sb.append(cb_)
        bcr_sb = []
        for k in range(3):
            b1 = const.tile([1, 4 * H], bf16, tag=f"bcr{k}")
            nc.scalar.dma_start(out=b1[:], in_=t_bcr[k].ap()[:, :])
            bcr_sb.append(b1)

        # dummy (+BIG) rows for the gather tables
        bigt = const.tile([1, H], bf16)
        nc.vector.memset(bigt[:], float(BIG))
        for ci in range(nseg[0]):
            nc.sync.dma_start(out=t_e1[ci].ap()[meta["e1_rows"][ci]:, :],
                              in_=bigt[:])
        nc.sync.dma_start(out=t_l2.ap()[meta["l2_rows"]:, :], in_=bigt[:])

        # resident gather indices (int16 wrapped layout), one tile per (k, ci)
        gidx_sb = {}
        for (k, ci), t in t_gidx.items():
            it = const.tile(list(t.shape), i16, tag=f"gidx{k}_{ci}")
            nc.sync.dma_start(out=it[:], in_=t.ap()[:, :])
            gidx_sb[(k, ci)] = it

        def leaky(out_ap, in_ap, ncols):
            """leaky(z) = max(z, 0.01 z); native Lrelu on Act when enabled."""
            if use_lrelu:
                nc.scalar.activation(out=out_ap, in_=in_ap,
                                     func=mybir.ActivationFunctionType.Lrelu,
                                     alpha=0.01)
            else:
                tt = ptmp.tile([128, 2 * H], bf16, tag="ttl")
                nc.scalar.mul(out=tt[:, :ncols], in_=in_ap, mul=0.01)
                nc.vector.tensor_tensor(out=out_ap, in0=in_ap,
                                        in1=tt[:, :ncols], op=ALU.max)

        tables = [None, t_e1, [t_l2]]
        outs = [t_e1, [t_l2], [t_f3]]

        def do_group(k, ci, R, gi, pst, rawl, gts):
            """One g-group: 128g dsts of conv k, class ci, bucket R.
            gts = SBUF slice [128, W*H] of gathered source rows (k>0)."""
            g = GMAP[R]
            W = g * R
            wx = wx_sb[k]
            base = row_bases[k][(ci, R)] + gi * g * 128
            out_t = outs[k][ci if k == 0 else 0]

            D = g * 128
            if k == 0:
                # transposed min-accumulate: zT = wv_chunk.T @ posSlot lands
                # [h-dims, slots] directly, so no PE transposes needed.
                mx = []
                for h2 in range(2):
                    if R == 1:
                        # single source: leaky straight off PSUM in one Act op
                        zTr = psA.tile([128, 512], f32, tag="psA")
                        nc.tensor.matmul(
                            out=zTr[:, :g * 128],
                            lhsT=wv_sb[:, h2 * 128:(h2 + 1) * 128],
                            rhs=pst[:, :g * 128],
                            start=True, stop=True)
                        mm = pmT.tile([128, g * 128], bf16, tag="mT")
                        leaky(mm[:], zTr[:, :g * 128], g * 128)
                        mx.append(mm)
                        continue
                    mta = pmta.tile([128, 512], bf16, tag="mTacc")
                    for r in range(R):
                        zTr = psA.tile([128, 512], f32, tag="psA")
                        nc.tensor.matmul(
                            out=zTr[:, :g * 128],
                            lhsT=wv_sb[:, h2 * 128:(h2 + 1) * 128],
                            rhs=pst[:, r * g * 128:(r + 1) * g * 128],
                            start=True, stop=True)
                        if r == 0:
                            # DVE can't read two PSUM operands; Act copy
                            nc.scalar.copy(out=mta[:, :g * 128],
                                           in_=zTr[:, :g * 128])
                        else:
                            nc.vector.tensor_tensor(out=mta[:, :g * 128],
                                                    in0=mta[:, :g * 128],
                                                    in1=zTr[:, :g * 128],
                                                    op=ALU.min)
                    # leaky on SBUF bf16: offloadable to gpsimd
                    leng = nc.gpsimd if gp_leaky else nc.vector
                    ttm = pbr.tile([128, g * 128], bf16, tag="ttm")
                    leng.tensor_scalar(out=ttm[:], in0=mta[:, :g * 128],
                                       scalar1=0.01, scalar2=None,
                                       op0=ALU.mult)
                    mm = pmT.tile([128, g * 128], bf16, tag="mT")
                    leng.tensor_tensor(out=mm[:], in0=mta[:, :g * 128],
                                       in1=ttm[:], op=ALU.max)
                    mx.append(mm)
            else:
                # min fold over rounds (col w = r*g + c keeps block assoc);
                # SBUF-only bf16 -> can run on the near-idle gpsimd engine
                feng = nc.gpsimd if gp_fold else nc.vector
                gt = gts
                s = R
                while s > 1:
                    h = s // 2
                    feng.tensor_tensor(
                        out=gt[:, : h * g * H], in0=gt[:, : h * g * H],
                        in1=gt[:, h * g * H: 2 * h * g * H], op=ALU.min)
                    if s % 2:
                        feng.tensor_tensor(
                            out=gt[:, : g * H], in0=gt[:, : g * H],
                            in1=gt[:, (s - 1) * g * H: s * g * H],
                            op=ALU.min)
                    s = h

            # x_dst transposed halves: xT = leaky(wx.T @ rawl)
            xT = []
            for h2 in range(2):
                zT = psA.tile([128, g * 128], f32, tag="psA")
                nc.tensor.matmul(out=zT[:],
                                 lhsT=wx[:, h2 * 128:(h2 + 1) * 128],
                                 rhs=rawl[:], start=True, stop=True)
                xs = pxT.tile([128, g * 128], bf16, tag="xT")
                leaky(xs[:], zT[:], g * 128)
                xT.append(xs)

            if k > 0:
                # mT via PE transpose of min columns (both halves one tile)
                zmT = psT.tile([128, 4 * H], bf16, tag="psT")
                for h2 in range(2):
                    for ci2 in range(g):
                        base_m = ci2 * H
                        nc.tensor.transpose(
                            out=zmT[:, (h2 * g + ci2) * 128:(h2 * g + ci2 + 1) * 128],
                            in_=gt[:, base_m + h2 * 128: base_m + (h2 + 1) * 128],
                            identity=ident[:])
                mmb = pmT.tile([128, 2 * g * 128], bf16, tag="mT")
                nc.vector.tensor_scalar(out=mmb[:], in0=zmT[:, :2 * g * 128],
                                        scalar1=1.0, scalar2=None,
                                        op0=ALU.mult)
                mx = [mmb[:, :g * 128], mmb[:, g * 128:2 * g * 128]]

            # x row-major via bf16 PE transpose of xT (independent of zo)
            px = psT.tile([128, 4 * H], bf16, tag="psT")
            for ci2 in range(g):
                for h2 in range(2):
                    nc.tensor.transpose(
                        out=px[:, ci2 * H + h2 * 128: ci2 * H + (h2 + 1) * 128],
                        in_=xT[h2][:, ci2 * 128:(ci2 + 1) * 128],
                        identity=ident[:])

            out_sb = pout.tile([128, g * H], f32 if k == 2 else bf16, tag="outC")
            for p2 in range(0, g, 2):
                pw = min(2, g - p2)
                # zo = x@S + m@(-B) + bc
                zo = psO.tile([128, 2 * H], f32, tag="psO")
                for c2 in range(pw):
                    ci2 = p2 + c2
                    cs = slice(ci2 * 128, (ci2 + 1) * 128)
                    zr = zo[:, c2 * H:(c2 + 1) * H]
                    nc.tensor.matmul(out=zr, lhsT=ones_row[:],
                                     rhs=bcr_sb[k][:, :H], start=True, stop=False)
                    nc.tensor.matmul(out=zr, lhsT=xT[0][:, cs], rhs=wS_sb[k][0][:],
                                     start=False, stop=False)
                    nc.tensor.matmul(out=zr, lhsT=xT[1][:, cs], rhs=wS_sb[k][1][:],
                                     start=False, stop=False)
                    nc.tensor.matmul(out=zr, lhsT=mx[0][:, cs], rhs=wnB_sb[k][0][:],
                                     start=False, stop=False)
                    nc.tensor.matmul(out=zr, lhsT=mx[1][:, cs], rhs=wnB_sb[k][1][:],
                                     start=False, stop=True)

                # out = x + leaky(zo)
                lk = ptmp.tile([128, 2 * H], bf16, tag="lk")
                leaky(lk[:, :pw * H], zo[:, :pw * H], pw * H)
                nc.vector.tensor_tensor(out=out_sb[:, p2 * H:(p2 + pw) * H],
                                        in0=px[:, p2 * H:(p2 + pw) * H],
                                        in1=lk[:, :pw * H],
                                        op=ALU.add)
            nc.sync.dma_start(
                out=out_t.ap()[base:base + g * 128, :]
                    .rearrange("(c p) d -> p c d", p=128),
                in_=out_sb[:].rearrange("p (c d) -> p c d", d=H))

        import os as _os
        _kset = [int(x) for x in _os.environ.get("BREP_KSET", "012")]
        for _rep in range(rep):
          for k in _kset:
            for ci in range(nseg[k]):
              for R in BUCKETS:
                  nb = bucket_counts[k].get((ci, R), 0)
                  if nb == 0:
                      continue
                  g = GMAP[R]
                  W = g * R
                  ngr = nb // g
                  cb = col_bases[k][(ci, R)]
                  # chunked staging loads: pslot (conv1) / rawT columns
                  ck_ps = max(1, 4096 // (W * 128))
                  ck_rw = max(1, 4096 // (g * 128))
                  ck = min(ck_ps, ck_rw) if k == 0 else ck_rw
                  # gather sub-chunk (groups per dma_gather)
                  gcg = max(1, GCOLS // W)
                  for g0 in range(0, ngr, ck):
                      n_in = min(ck, ngr - g0)
                      if k == 0:
                          pch = pps.tile([4, ck * W * 128], bf16, tag="pslot")
                          nc.sync.dma_start(
                              out=pch[:, :n_in * W * 128]
                                  .rearrange("p (n w) -> p n w", w=W * 128),
                              in_=t_pslot[(ci, R)].ap()[g0:g0 + n_in]
                                  .rearrange("n p w -> p n w"))
                      rch = praw.tile([Fs[k], ck * g * 128], bf16, tag="rawC")
                      ccb = (cb + g0 * g) * 128
                      nc.sync.dma_start(
                          out=rch[:, :n_in * g * 128],
                          in_=t_rawT[k].ap()[:, ccb:ccb + n_in * g * 128])
                      if k > 0:
                          src_t = tables[k][ci if k == 1 else 0]
                          gcb = gcol_bases[k][(ci, R)]
                          gchunks = {}
                          for gg in range(g0, g0 + n_in, gcg):
                              n_g = min(gcg, g0 + n_in - gg)
                              cols = n_g * W
                              gt = pgath.tile([128, 32 * H], bf16, tag="g")
                              c0 = gcb + gg * W
                              # single_packet=False: packed-stream mode wedges
                              # the device beyond ~64 descriptors per ring
                              nc.gpsimd.dma_gather(
                                  gt[:, :cols * H]
                                      .rearrange("p (g e) -> p g e", e=H),
                                  src_t.ap()[:, :],
                                  gidx_sb[(k, ci)][:, c0 * 8:(c0 + cols) * 8],
                                  cols * 128, cols * 128, H,
                                  single_packet=False)
                              gchunks[gg] = gt
                      for gi in range(g0, g0 + n_in):
                          off = gi - g0
                          pst = (pch[:, off * W * 128:(off + 1) * W * 128]
                                 if k == 0 else None)
                          rawl = rch[:, off * g * 128:(off + 1) * g * 128]
                          gts = None
                          if k > 0:
                              gg = g0 + ((gi - g0) // gcg) * gcg
                              goff = gi - gg
                              gts = gchunks[gg][:, goff * W * H:(goff + 1) * W * H]
                          do_group(k, ci, R, gi, pst, rawl, gts)

    nc.compile()
    return nc


# ==========================================================================
# Entry point
# ==========================================================================

def kernel(**inputs):
    import os
    meta, per_core_inputs, per_core_rowmaps, cores = _host_prep(inputs)
    nc = _build_kernel(meta,
                       use_lrelu=not os.environ.get("BREP_NO_LRELU"))

    in_maps = [dict(im) for im in per_core_inputs]

    if os.environ.get("BREP_SIM"):
        from concourse.bass_interp import CoreSim
        nc_sim = _build_kernel(meta, use_lrelu=False)  # interp can't exec Lrelu
        results = []
        for ci in range(int(os.environ.get("BREP_SIM_CORES", C))):
            sim = CoreSim(nc_sim, trace=False)
            for name, arr in in_maps[ci].items():
                sim.tensor(name)[:] = arr
            sim.simulate()
            results.append({"f3": np.array(sim.tensor("f3"))})
    else:
        res = run_bass_kernel_spmd(nc, in_maps, core_ids=list(range(C)))
        results = res.results

    NF = inputs["face_surfaces"].shape[0]
    out = np.empty((NF, H), np.float32)
    for ci, (r, c) in enumerate(zip(results, cores)):
        f3 = r["f3"]
        rm = per_core_rowmaps[ci][2]          # local face -> table row
        out[c["lo"]:c["hi"]] = f3[rm]
    return out
